# revision 1
# baseline (speedup 1.0000x reference)
"""AGCRN cell on 8 Trainium2 NeuronCores.

Sharding: batch B=64 is split 8 ways across cores (data parallel).
node_embeddings and the weight/bias pools are replicated; each core
computes the NxN supports locally and its batch-slice of the output.
Inputs are the FULL arrays; output is the FULL (B, N, HIDDEN) array.
"""

import numpy as np
import jax
import jax.numpy as jnp
from jax.sharding import Mesh, PartitionSpec as P

try:  # jax>=0.4.35 moved shard_map out of experimental
    from jax.experimental.shard_map import shard_map
except ImportError:  # pragma: no cover
    from jax.shard_map import shard_map

CHEB_K = 2
HIDDEN = 64
N_CORES = 8

# Hardcoded problem shapes (nn_AGCRNCell_3616362463697)
B, N, C_IN, D = 64, 2048, 2, 16
C = C_IN + HIDDEN  # 66


def _avwgcn(x, E, Wp, bp):
    """x: (b, N, C), E: (N, D), Wp: (D, K, C, O), bp: (D, O) -> (b, N, O)."""
    supports = jax.nn.softmax(jax.nn.relu(E @ E.T), axis=1)  # (N, N)
    # Cheb K=2: support set is [I, supports]
    x1 = jnp.einsum('nm,bmc->bnc', supports, x)  # (b, N, C)
    weights = jnp.einsum('nd,dkio->nkio', E, Wp)  # (N, K, C, O)
    bias = E @ bp  # (N, O)
    out = (
        jnp.einsum('bnc,nco->bno', x, weights[:, 0])
        + jnp.einsum('bnc,nco->bno', x1, weights[:, 1])
        + bias
    )
    return out


def _cell(x, state, E, gw, gb, uw, ub):
    inp = jnp.concatenate([x, state], axis=-1)
    zr = jax.nn.sigmoid(_avwgcn(inp, E, gw, gb))
    z, r = zr[..., :HIDDEN], zr[..., HIDDEN:]
    cand = jnp.concatenate([x, r * state], axis=-1)
    hc = jnp.tanh(_avwgcn(cand, E, uw, ub))
    return z * state + (1.0 - z) * hc


_compiled = None


def _get_compiled():
    global _compiled
    if _compiled is not None:
        return _compiled
    devs = jax.devices()[:N_CORES]
    mesh = Mesh(np.asarray(devs), ('x',))
    fn = shard_map(
        _cell,
        mesh=mesh,
        in_specs=(P('x'), P('x'), P(), P(), P(), P(), P()),
        out_specs=P('x'),
    )
    _compiled = jax.jit(fn)
    return _compiled


def kernel(x, state, node_embeddings, gate_w, gate_b, update_w, update_b):
    fn = _get_compiled()
    out = fn(
        jnp.asarray(x, jnp.float32),
        jnp.asarray(state, jnp.float32),
        jnp.asarray(node_embeddings, jnp.float32),
        jnp.asarray(gate_w, jnp.float32),
        jnp.asarray(gate_b, jnp.float32),
        jnp.asarray(update_w, jnp.float32),
        jnp.asarray(update_b, jnp.float32),
    )
    return np.asarray(jax.device_get(out), dtype=np.float32)

